# revision 5
# baseline (speedup 1.0000x reference)
"""Trainium2 Bass kernel for nn_Canvas_DIP_by_distance (vq_codebook), v4.

reference semantics:
  weight = sigmoid(weight_logits)                       (224, 224, 3)
  d[h,w,c] = sum_k (palette[c,k] - weight[h,w,k])^2     (224, 224, 64)
  idx = argmax_c softmax(d + 1) = argmax_c d
  colors[ch,h,w] = palette[idx[h,w], ch]                (3, 224, 224)
  out = nearest_upsample(colors, 2048, 2048)            (3, 2048, 2048)

v4 design (per core: 28 canvas rows -> 256 output rows):
  - argmax front per quarter (7 canvas rows) exactly as v2/v3: fp32
    distances via block-diag matmul, 8x8-factorized one-hot on DVE,
    palette apply via transpose + block-diag matmul + DVE select.
  - column expansion per quarter: full-slot matmul [124, 512] x 4 ccs
    (single wf each: cols < 1024 only ever read w < 112), then copy ONLY
    this quarter's 28-partition region into exp32 (fp32, [128, 2048]).
  - row replication is done BY THE STORE DMAs: for each (quarter, ch)
    one dma_start reads 7 slot partitions and writes each 2048-px row 9
    times (stride-0 broadcast middle axis); rows 64g (the 10th copy of
    hh=7g) go in small tail DMAs. No replicate matmuls, no [128, 512]
    copies, no ofs buffer.
  - quarter g sits at partitions P(g) + 4j + ch with P = [0, 64, 32,
    96]: consecutive quarters alternate even/odd SDMA port parity so
    both port groups stream concurrently.

slot layout: partition = P[g] + 4j + ch for canvas row hh = 7g + j.
"""

import numpy as np
from contextlib import ExitStack

CANVAS_H, CANVAS_W, NUM_COLORS = 224, 224, 64
IMAGE_H = IMAGE_W = 2048
N_CORES = 8
HC = CANVAS_H // N_CORES          # 28 canvas rows per core
ORC = IMAGE_H // N_CORES          # 256 output rows per core
WH = CANVAS_W // 2                # 112
PQ = [0, 64, 32, 96]              # quarter -> partition base

_CACHE = {}


def _build_program():
    import concourse.bacc as bacc
    import concourse.tile as tile
    import concourse.mybir as mybir
    from concourse import bass

    f32 = mybir.dt.float32
    f16 = mybir.dt.float16
    ALU = mybir.AluOpType
    nc = bacc.Bacc("TRN2", target_bir_lowering=False)

    w4g_in = nc.dram_tensor("w4g_in", [28, 2, 4, 112], f32, kind="ExternalInput")
    b4c_in = nc.dram_tensor("b4c_in", [28, 448], f32, kind="ExternalInput")
    p2e_in = nc.dram_tensor("p2e_in", [56, 168], f16, kind="ExternalInput")
    id16_in = nc.dram_tensor("id16_in", [112, 112], f16, kind="ExternalInput")
    esb_in = nc.dram_tensor("esb_in", [112, 2, 1024], f16, kind="ExternalInput")
    out = nc.dram_tensor("out", [3, ORC, IMAGE_W], f32, kind="ExternalOutput")

    with tile.TileContext(nc) as tc:
        with ExitStack() as ctx:
            sb = ctx.enter_context(tc.tile_pool(name="sb", bufs=1))
            ps = ctx.enter_context(tc.tile_pool(name="ps", bufs=1, space="PSUM"))

            # ---- const loads on the scalar ring (sync ring = stores only)
            w4g = sb.tile([28, 2, 4, 112], f32, tag="w4g")
            nc.scalar.dma_start(out=w4g[:], in_=w4g_in[:])
            b4c = sb.tile([28, 448], f32, tag="b4c")
            nc.scalar.dma_start(out=b4c[:], in_=b4c_in[:])
            p2e = sb.tile([56, 168], f16, tag="p2e")
            nc.scalar.dma_start(out=p2e[:], in_=p2e_in[:])
            id16 = sb.tile([112, 112], f16, tag="id16")
            nc.scalar.dma_start(out=id16[:], in_=id16_in[:])
            esb = sb.tile([112, 2, 1024], f16, tag="esb")
            nc.scalar.dma_start(out=esb[:], in_=esb_in[:])

            colors = sb.tile([112, 2, 124], f16, tag="colors")
            nc.gpsimd.memset(colors[:], 0.0)

            exp32 = sb.tile([128, 2048], f32, tag="exp32")

            # PSUM budget (8 banks): vps 3x1 + tps 1 + m1 1 + eps 2x1 = 7
            def front(q):
                """quarter q (canvas rows 7q..7q+6) -> colors slots."""
                m8a = sb.tile([112, 2, 7, 8], f32, tag="m8a", bufs=2)
                m8b = sb.tile([112, 2, 7, 8], f32, tag="m8b", bufs=2)
                for wf in range(2):
                    vq = ps.tile([112, 448], f32, tag="vps", bufs=3)
                    nc.tensor.matmul(
                        out=vq[:], lhsT=w4g[:, wf, q],
                        rhs=b4c[:], start=True, stop=True)
                    nc.vector.tensor_reduce(
                        out=m8a[:, wf],
                        in_=vq[:].rearrange("w (j a b) -> w j a b", a=8, b=8),
                        axis=mybir.AxisListType.X, op=ALU.max)
                    nc.vector.tensor_reduce(
                        out=m8b[:, wf],
                        in_=vq[:].rearrange("w (j a b) -> w j b a", a=8, b=8),
                        axis=mybir.AxisListType.X, op=ALU.max)
                vmax = sb.tile([112, 2, 7], f32, tag="vmax", bufs=2)
                nc.vector.tensor_reduce(
                    out=vmax[:], in_=m8a[:], axis=mybir.AxisListType.X,
                    op=ALU.max)
                vmb = vmax[:].unsqueeze(3).to_broadcast([112, 2, 7, 8])
                oha = sb.tile([112, 2, 7, 8], f16, tag="oha", bufs=2)
                nc.vector.tensor_tensor(
                    out=oha[:], in0=m8a[:], in1=vmb, op=ALU.is_equal)
                ohb = sb.tile([112, 2, 7, 8], f16, tag="ohb", bufs=2)
                nc.vector.tensor_tensor(
                    out=ohb[:], in0=m8b[:], in1=vmb, op=ALU.is_equal)
                m1 = ps.tile([112, 2, 256], f32, tag="m1ps", bufs=1)
                for wf in range(2):
                    tps = ps.tile([56, 112], f16, tag="tps", bufs=1)
                    nc.tensor.transpose(
                        out=tps[:],
                        in_=oha[:, wf].rearrange("w j a -> w (j a)"),
                        identity=id16[:, 0:112])
                    ohaT = sb.tile([56, 112], f16, tag="ohaT", bufs=2)
                    nc.scalar.copy(out=ohaT[:], in_=tps[:])
                    nc.tensor.matmul(
                        out=m1[:, wf, 0:168], lhsT=ohaT[:], rhs=p2e[:],
                        start=True, stop=True)
                tmp = sb.tile([112, 2, 7, 3, 8], f16, tag="tmp", bufs=2)
                nc.vector.tensor_tensor(
                    out=tmp[:],
                    in0=m1[:, :, 0:168].rearrange(
                        "w f (j c b) -> w f j c b", c=3, b=8),
                    in1=ohb[:].unsqueeze(3).to_broadcast([112, 2, 7, 3, 8]),
                    op=ALU.mult)
                p0 = PQ[q]
                cdst = (colors[:, :, p0:p0 + 28]
                        .rearrange("w f (j s) -> w f j s", s=4)[:, :, :, 0:3])
                with nc.allow_low_precision(
                        reason="one-hot select: sum has a single nonzero f16"):
                    nc.vector.tensor_reduce(
                        out=cdst, in_=tmp[:], axis=mybir.AxisListType.X,
                        op=ALU.add)

            def expand_store(g):
                """column-expand quarter g's region + broadcast-store it."""
                p0 = PQ[g]
                for cc in range(4):
                    wf = cc // 2
                    eps = ps.tile([124, 512], f32, tag="eps", bufs=2)
                    nc.tensor.matmul(
                        out=eps[:], lhsT=colors[:, wf, 0:124],
                        rhs=esb[:, wf, 512 * (cc % 2):512 * (cc % 2) + 512],
                        start=True, stop=True)
                    nc.scalar.copy(
                        out=exp32[p0:p0 + 28, 512 * cc:512 * cc + 512],
                        in_=eps[p0:p0 + 28, :])
                for ch in range(3):
                    src = (exp32[p0 + ch:p0 + ch + 28:4, :]
                           .unsqueeze(1).to_broadcast([7, 9, 2048]))
                    dst = (out[ch, 64 * g + 1:64 * g + 64, :]
                           .rearrange("(j r) c -> j r c", r=9))
                    nc.sync.dma_start(out=dst, in_=src)

            def tail_stores():
                """rows 64g (10th copy of hh=7g) for all quarters/channels."""
                for ch in range(3):
                    for par in range(2):
                        # partitions {ch+64*par, ch+64*par+32}
                        #   -> rows {128*?, ...}: P=[0,64,32,96] so
                        # par=0: parts {ch, ch+32}   -> rows {0, 128}
                        # par=1: parts {ch+64,ch+96} -> rows {64, 192}
                        src = exp32[64 * par + ch:64 * par + ch + 33:32, :]
                        dst = (out[ch, :, :]
                               .rearrange("(b a r) c -> b a r c",
                                          b=2, a=2, r=64)[:, par, 0])
                        nc.sync.dma_start(out=dst, in_=src)

            front(0)
            expand_store(0)
            front(1)
            expand_store(1)
            front(2)
            expand_store(2)
            front(3)
            expand_store(3)
            tail_stores()

    nc.compile()
    return nc


def _host_consts(weight_logits: np.ndarray, palette: np.ndarray):
    """Build per-core input tensors (host does sigmoid + layouts)."""
    pal = palette.astype(np.float32)
    pal16 = pal.astype(np.float16)
    sig = (1.0 / (1.0 + np.exp(-weight_logits.astype(np.float64))))
    sig = sig.astype(np.float32)                      # (224, 224, 3)

    # b4c [28=(7j 4k), 448=(7j 64c)] block-diagonal
    b4row = np.empty((4, NUM_COLORS), np.float32)
    b4row[0:3] = -pal.T
    b4row[3] = 0.5 * (pal.astype(np.float64) ** 2).sum(-1).astype(np.float32)
    b4c = np.zeros((28, 448), np.float32)
    for j in range(7):
        b4c[4 * j:4 * j + 4, 64 * j:64 * j + 64] = b4row

    # p2e [56=(7j 8a), 168=(7j 3ch 8b)] block-diagonal
    p2 = pal16.reshape(8, 8, 3)                       # [a, b, ch]
    blk = np.transpose(p2, (0, 2, 1)).reshape(8, 24)  # [a, (ch b)]
    p2e = np.zeros((56, 168), np.float16)
    for j in range(7):
        p2e[8 * j:8 * j + 8, 24 * j:24 * j + 24] = blk

    # esb [112, 2, 1024]: wf-split 0/1 column-expansion.
    # cols 0..1023 only read w<112 (wf 0); cols 1024..2047 only w>=112.
    wmap = (np.arange(IMAGE_W) * CANVAS_W) // IMAGE_W
    e_full = (wmap[None, :] == np.arange(CANVAS_W)[:, None]).astype(np.float16)
    esb = np.ascontiguousarray(
        np.stack([e_full[:WH, 0:1024], e_full[WH:, 1024:2048]], axis=1))

    id16 = np.eye(112, dtype=np.float16)

    # per-core w4g [112=(4q 7j 4k), 2, 112]
    w4gs = []
    for core in range(N_CORES):
        s = sig[core * HC:(core + 1) * HC]            # (28, 224, 3)
        w4g = np.empty((28, 2, 4, 112), np.float32)
        for q in range(4):
            for j in range(7):
                row = s[7 * q + j]                    # (224, 3)
                for k in range(4):
                    v = (row[:, k] if k < 3
                         else np.ones(224, np.float32))
                    w4g[4 * j + k, 0, q] = v[:WH]
                    w4g[4 * j + k, 1, q] = v[WH:]
        w4gs.append(np.ascontiguousarray(w4g))

    return w4gs, b4c, p2e, esb, id16


def make_in_maps(weight_logits, palette):
    w4gs, b4c, p2e, esb, id16 = _host_consts(weight_logits, palette)
    in_maps = []
    for core in range(N_CORES):
        in_maps.append({
            "w4g_in": w4gs[core], "b4c_in": b4c, "p2e_in": p2e,
            "id16_in": id16, "esb_in": esb,
        })
    return in_maps


def kernel(weight_logits, palette, image_h, image_w):
    weight_logits = np.asarray(weight_logits, np.float32)
    palette = np.asarray(palette, np.float32)
    assert int(image_h) == IMAGE_H and int(image_w) == IMAGE_W
    assert weight_logits.shape == (CANVAS_H, CANVAS_W, 3)

    if "nc" not in _CACHE:
        _CACHE["nc"] = _build_program()
    nc = _CACHE["nc"]

    from concourse import bass_utils

    res = bass_utils.run_bass_kernel_spmd(
        nc, make_in_maps(weight_logits, palette),
        core_ids=list(range(N_CORES)))
    outs = [res.results[c]["out"] for c in range(N_CORES)]
    return np.concatenate(outs, axis=1)
